# revision 2
# baseline (speedup 1.0000x reference)
"""Trainium2 Bass kernel for nn_LoraLinear (embedding_lookup, 8 cores).

Computation (per batch row b):
    out[b] = x[b] @ W_base.T + b_base
             + S * ( (B_user[u_b] + B_item[i_b] + W_common) @ (x[b] @ A.T) )
with S = 2.0, shapes: x [4096,1024], tables [10000,1024,16], A [16,1024],
W_common [1024,16], out [4096,1024].

Strategy: pure data-parallel over the batch (512 rows/core); B tables are
replicated in each core's HBM (bf16) and rows are fetched with indirect
DMA gathers. The rank-16 per-row "matvec" is computed on the TensorEngine
as block-diagonal matmuls: 64 batch rows are packed per matmul with the
contraction dim = 64 rows x 2 r-halves = 128 partitions; the r-sum is
completed by accumulating 8 such matmuls (8 r-chunks) into PSUM. The base
matmul (fp32), bias (K=1 matmul) and common part accumulate into the same
PSUM banks. No collectives.

Host-side prep (not on the accelerator): layout transposes, bf16 cast of
the tables, and index arithmetic (idx -> macro-row ids) only.
"""
import numpy as np
import ml_dtypes

import concourse.bass as bass
import concourse.bacc as bacc
import concourse.tile as tile
from concourse import mybir
from concourse.bass_utils import run_bass_kernel_spmd

# problem shapes (hardcoded per contract)
IN_F = 1024
OUT_F = 1024
R = 16
NUM_USERS = 10000
NUM_ITEMS = 10000
BATCH = 4096
SCALING = 2.0
N_CORES = 8

B_SH = BATCH // N_CORES          # 512 rows per core
RG = 64                          # batch rows packed per matmul group
S_SUB = 2                        # r-halves per partition dim (RG * S_SUB = 128)
C_SUB = R // S_SUB               # 8 r-chunks accumulated via separate matmuls
NG = B_SH // RG                  # 8 groups per core
NKC = IN_F // 128                # 8 contraction chunks for the base matmul
NH = OUT_F // 512                # 2 output halves (PSUM bank free-dim limit)
NBG = B_SH // 128                # 4 PSUM row-blocks

F32 = mybir.dt.float32
BF16 = mybir.dt.bfloat16
I32 = mybir.dt.int32

_CACHE = {}


def _build():
    nc = bacc.Bacc("TRN2", target_bir_lowering=False, debug=False,
                   num_devices=N_CORES)
    xt = nc.dram_tensor("xt", [IN_F, B_SH], F32, kind="ExternalInput")
    wt = nc.dram_tensor("wt", [IN_F, OUT_F], F32, kind="ExternalInput")
    a2w = nc.dram_tensor("a2w", [128, NKC * R], F32, kind="ExternalInput")
    wct = nc.dram_tensor("wct", [R, OUT_F], F32, kind="ExternalInput")
    biasb = nc.dram_tensor("biasb", [1, OUT_F], BF16, kind="ExternalInput")
    ones1 = nc.dram_tensor("ones1", [1, 128], BF16, kind="ExternalInput")
    ltab = nc.dram_tensor("ltab", [R, C_SUB * 128], F32, kind="ExternalInput")
    masks = nc.dram_tensor("masks", [128, RG], F32, kind="ExternalInput")
    but = nc.dram_tensor("but", [NUM_USERS * S_SUB, C_SUB * OUT_F], BF16,
                         kind="ExternalInput")
    bit = nc.dram_tensor("bit", [NUM_ITEMS * S_SUB, C_SUB * OUT_F], BF16,
                         kind="ExternalInput")
    uidx = nc.dram_tensor("uidx", [128, NG], I32, kind="ExternalInput")
    iidx = nc.dram_tensor("iidx", [128, NG], I32, kind="ExternalInput")
    y = nc.dram_tensor("y", [B_SH, OUT_F], F32, kind="ExternalOutput")

    with tile.TileContext(nc) as tc:
        with (
            tc.tile_pool(name="const", bufs=1) as cp,
            tc.tile_pool(name="gath", bufs=4) as gp,
            tc.tile_pool(name="btp", bufs=16) as btp,
            tc.tile_pool(name="ps", bufs=8, space="PSUM") as psp,
            tc.tile_pool(name="outp", bufs=3) as op,
        ):
            # ---- constant / weight loads ----
            xt_t = []
            for k in range(NKC):
                t = cp.tile([128, B_SH], F32, tag=f"xt{k}")
                nc.sync.dma_start(t[:], xt.ap()[128 * k:128 * (k + 1), :])
                xt_t.append(t)
            wt_t = []
            for k in range(NKC):
                t = cp.tile([128, OUT_F], F32, tag=f"wt{k}")
                nc.sync.dma_start(t[:], wt.ap()[128 * k:128 * (k + 1), :])
                wt_t.append(t)
            a2w_t = cp.tile([128, NKC * R], F32, tag="a2w")
            nc.sync.dma_start(a2w_t[:], a2w.ap())
            wct_t = cp.tile([R, OUT_F], F32, tag="wct")
            nc.sync.dma_start(wct_t[:], wct.ap())
            bias_t = cp.tile([1, OUT_F], BF16, tag="bias")
            nc.sync.dma_start(bias_t[:], biasb.ap())
            ones_t = cp.tile([1, 128], BF16, tag="ones")
            nc.sync.dma_start(ones_t[:], ones1.ap())
            ltab_t = cp.tile([R, C_SUB * 128], F32, tag="ltab")
            nc.sync.dma_start(ltab_t[:], ltab.ap())
            mask_t = cp.tile([128, RG], F32, tag="mask")
            nc.sync.dma_start(mask_t[:], masks.ap())
            uidx_t = cp.tile([128, NG], I32, tag="uidx")
            nc.sync.dma_start(uidx_t[:], uidx.ap())
            iidx_t = cp.tile([128, NG], I32, tag="iidx")
            nc.sync.dma_start(iidx_t[:], iidx.ap())

            # ---- a2T = (2A) @ x_shard.T  -> [16, 512] f32 ----
            a2t_ps = psp.tile([R, B_SH], F32, tag="ps", space="PSUM")
            for k in range(NKC):
                nc.tensor.matmul(
                    a2t_ps[:], lhsT=a2w_t[:, R * k:R * (k + 1)], rhs=xt_t[k][:],
                    start=(k == 0), stop=(k == NKC - 1), skip_group_check=True)
            a2t_sb = cp.tile([R, B_SH], F32, tag="a2t")
            nc.vector.tensor_copy(a2t_sb[:], a2t_ps[:])

            # ---- rep_c[p, col] = a2T[S*(p%S)+c... (see ltab), one per r-chunk ----
            rep_sb = []
            for c in range(C_SUB):
                rps = psp.tile([128, B_SH], F32, tag="ps", space="PSUM")
                nc.tensor.matmul(
                    rps[:], lhsT=ltab_t[:, 128 * c:128 * (c + 1)], rhs=a2t_sb[:],
                    start=True, stop=True, skip_group_check=True)
                rsb = cp.tile([128, B_SH], F32, tag=f"rep{c}")
                nc.vector.tensor_copy(rsb[:], rps[:])
                rep_sb.append(rsb)

            # ---- output PSUM banks + bias/base/common accumulation ----
            out_ps = {}
            for bg in range(NBG):
                for h in range(NH):
                    ps = psp.tile([128, 512], F32, tag="ps", space="PSUM")
                    out_ps[(bg, h)] = ps
                    nc.tensor.matmul(  # bias broadcast (K=1)
                        ps[:], lhsT=ones_t[:], rhs=bias_t[:, 512 * h:512 * h + 512],
                        start=True, stop=False, skip_group_check=True)
                    for k in range(NKC):  # base: x @ W_base.T (fp32)
                        nc.tensor.matmul(
                            ps[:], lhsT=xt_t[k][:, 128 * bg:128 * (bg + 1)],
                            rhs=wt_t[k][:, 512 * h:512 * h + 512],
                            start=False, stop=False, skip_group_check=True)
                    nc.tensor.matmul(  # common: a2 @ W_common.T
                        ps[:], lhsT=a2t_sb[:, 128 * bg:128 * (bg + 1)],
                        rhs=wct_t[:, 512 * h:512 * h + 512],
                        start=False, stop=False, skip_group_check=True)

            # ---- lora gathers + block-diagonal matmuls ----
            n_left = {k: 2 * 2 * C_SUB for k in out_ps}  # matmuls per bank
            for g in range(NG):
                gpb = 128 // RG          # groups per 128-row PSUM bank
                bg, strip = g // gpb, (g % gpb) * RG
                bts = []
                for c in range(C_SUB):
                    bt = btp.tile([128, RG], BF16, tag="bt")
                    nc.vector.tensor_tensor(
                        out=bt[:], in0=mask_t[:],
                        in1=rep_sb[c][:, RG * g:RG * (g + 1)],
                        op=mybir.AluOpType.mult)
                    bts.append(bt)
                for tab_ap, idx_t in ((but.ap(), uidx_t), (bit.ap(), iidx_t)):
                    gt = gp.tile([128, C_SUB * OUT_F], BF16, tag="gt")
                    nc.gpsimd.indirect_dma_start(
                        out=gt[:], out_offset=None, in_=tab_ap,
                        in_offset=bass.IndirectOffsetOnAxis(
                            ap=idx_t[:, g:g + 1], axis=0))
                    for c in range(C_SUB):
                        for h in range(NH):
                            key = (bg, h)
                            n_left[key] -= 1
                            nc.tensor.matmul(
                                out_ps[key][strip:strip + RG, :],
                                lhsT=bts[c][:],
                                rhs=gt[:, OUT_F * c + 512 * h:OUT_F * c + 512 * h + 512],
                                start=False, stop=(n_left[key] == 0),
                                tile_position=(0, strip),
                                skip_group_check=True)

            # ---- PSUM -> SBUF -> DRAM ----
            for bg in range(NBG):
                for h in range(NH):
                    ot = op.tile([128, 512], F32, tag="ot")
                    nc.scalar.copy(ot[:], out_ps[(bg, h)][:])
                    nc.sync.dma_start(
                        y.ap()[128 * bg:128 * (bg + 1), 512 * h:512 * h + 512],
                        ot[:])
    nc.compile()
    return nc


def _prep_host(x, user_indices, item_indices, W_base, b_base, A, B_user,
               B_item, W_common):
    """Host-side layout prep. Returns (shared dict, per-core list of dicts)."""
    bf16 = ml_dtypes.bfloat16
    x = np.asarray(x, np.float32)
    W_base = np.asarray(W_base, np.float32)
    b_base = np.asarray(b_base, np.float32)
    A = np.asarray(A, np.float32)
    W_common = np.asarray(W_common, np.float32)
    user_indices = np.asarray(user_indices, np.int32)
    item_indices = np.asarray(item_indices, np.int32)

    wt = np.ascontiguousarray(W_base.T)                       # [in, out]
    a2t = np.ascontiguousarray((SCALING * A).T)               # [in, R]
    # a2w[p, R*k + r] = a2t[128k + p, r]
    a2w = np.ascontiguousarray(
        a2t.reshape(NKC, 128, R).transpose(1, 0, 2).reshape(128, NKC * R))
    wct = np.ascontiguousarray(W_common.T)                    # [R, out]
    biasb = b_base.reshape(1, OUT_F).astype(bf16)
    ones1 = np.ones((1, 128), bf16)
    # ltab[r, 128c + p] = 1 if r == C_SUB*(p % S_SUB) + c
    ltab = np.zeros((R, C_SUB * 128), np.float32)
    p = np.arange(128)
    for c in range(C_SUB):
        ltab[C_SUB * (p % S_SUB) + c, 128 * c + p] = 1.0
    # masks[p, j] = 1 if p // S_SUB == j
    masks = np.zeros((128, RG), np.float32)
    masks[p, p // S_SUB] = 1.0
    # tables: [U, out, R] -> [U, R, out] bf16 -> macro rows [U*S, C*out]
    but = np.ascontiguousarray(np.asarray(B_user, np.float32).transpose(0, 2, 1)) \
        .astype(bf16).reshape(NUM_USERS * S_SUB, C_SUB * OUT_F)
    bit = np.ascontiguousarray(np.asarray(B_item, np.float32).transpose(0, 2, 1)) \
        .astype(bf16).reshape(NUM_ITEMS * S_SUB, C_SUB * OUT_F)

    shared = dict(wt=wt, a2w=a2w, wct=wct, biasb=np.asarray(biasb),
                  ones1=np.asarray(ones1), ltab=ltab, masks=masks,
                  but=np.asarray(but), bit=np.asarray(bit))
    per_core = []
    s = np.arange(128) % S_SUB
    i64 = np.arange(128) // S_SUB
    for c in range(N_CORES):
        sl = slice(B_SH * c, B_SH * (c + 1))
        xt_c = np.ascontiguousarray(x[sl].T)                  # [in, 512]
        u = user_indices[sl]
        it = item_indices[sl]
        uidx = np.empty((128, NG), np.int32)
        iidx = np.empty((128, NG), np.int32)
        for g in range(NG):
            uidx[:, g] = S_SUB * u[RG * g + i64] + s
            iidx[:, g] = S_SUB * it[RG * g + i64] + s
        per_core.append(dict(xt=xt_c, uidx=uidx, iidx=iidx))
    return shared, per_core


def kernel(**inputs) -> np.ndarray:
    if "nc" not in _CACHE:
        _CACHE["nc"] = _build()
    nc = _CACHE["nc"]
    shared, per_core = _prep_host(**inputs)
    in_maps = [{**shared, **pc} for pc in per_core]
    res = run_bass_kernel_spmd(nc, in_maps, core_ids=list(range(N_CORES)))
    out = np.concatenate([res.results[c]["y"] for c in range(N_CORES)], axis=0)
    return out.astype(np.float32)
